# revision 21
# baseline (speedup 1.0000x reference)
"""Causal single-head attention on 8 trn2 NeuronCores, data-parallel over batch.

Per core (one batch element, C=2048 ctx, E=1024 emb, D=1024 query_dim):
  P_X: cast x to fp16 (DVE), PE-transpose fp16 -> xt resident SBUF,
       interleaved chunk-wise with P_Q so the PE never idles at startup.
  P_Q: qT = (Qw^T @ x^T) + Qb   -> resident SBUF (DT), [d-chunk][128, C].
  P_K: kT = (Kw^T @ x^T) + Kb   -> resident SBUF (DT), same layout.
  P_V: v  = (x @ Vw)            -> resident SBUF (DT), natural layout;
       Vb is folded into the epilogue: out = (E^T V)/rowsum + Vb.
  A:   software-pipelined over 128-row query blocks: scores_{i+1} is issued
       before transpose_i/out_i so the PE never waits on the scalar-engine
       exp. Per block i: scores = qT_i^T @ kT (causal range only), additive
       -1e9 mask on the diagonal tile, E = exp(scale*scores) with fused
       row-sum (ACT), PE-transpose E, psum->sbuf E^T copies on ACT,
       out = sum_j E^T_j @ v_j in PSUM, epilogue: DVE scale by 1/rowsum,
       GPSIMD adds broadcast Vb, DMA out.
"""

import os
import sys

for _p in ("/opt/trn_rl_repo", "/root/.axon_site/_ro/trn_rl_repo"):
    if os.path.isdir(_p) and _p not in sys.path:
        sys.path.insert(0, _p)

from contextlib import ExitStack

import numpy as np

import concourse.bass as bass
import concourse.tile as tile
from concourse import bacc, mybir
from concourse.masks import make_causal_mask, make_identity

F32 = mybir.dt.float32
AF = mybir.ActivationFunctionType
DTYPES = {"fp16": mybir.dt.float16, "bf16": mybir.dt.bfloat16}

P = 128


def build(C=2048, E=1024, D=1024, n_cores=8, loop=1, dt="fp16", marks=None,
          knobs=None):
    DT = DTYPES[dt]
    CC = 512            # c-chunk width for projection passes
    NJ = 512            # matmul moving-dim width
    NCC = C // CC
    EC = E // P         # contraction chunks for projections
    DC = D // P
    RB = C // P         # number of 128-row blocks
    ND = D // NJ
    NKC = C // NJ
    scale = float(D) ** -0.5

    nc = bacc.Bacc("TRN2", target_bir_lowering=False, debug=False,
                   num_devices=n_cores)
    x_d = nc.dram_tensor("x", [C, E], F32, kind="ExternalInput").ap()
    qw_d = nc.dram_tensor("Qw", [E, D], F32, kind="ExternalInput").ap()
    qb_d = nc.dram_tensor("Qb", [D], F32, kind="ExternalInput").ap()
    kw_d = nc.dram_tensor("Kw", [E, D], F32, kind="ExternalInput").ap()
    kb_d = nc.dram_tensor("Kb", [D], F32, kind="ExternalInput").ap()
    vw_d = nc.dram_tensor("Vw", [E, D], F32, kind="ExternalInput").ap()
    vb_d = nc.dram_tensor("Vb", [D], F32, kind="ExternalInput").ap()
    out_d = nc.dram_tensor("out", [C, D], F32, kind="ExternalOutput").ap()

    def mark(label):
        if marks is not None:
            marks.append((label, nc.next_id()))

    with tile.TileContext(nc) as tc, ExitStack() as ctx:
        if loop > 1:
            ctx.enter_context(tc.For_i(0, loop, 1))
        const_pool = ctx.enter_context(tc.tile_pool(name="const", bufs=1))
        v_pool = ctx.enter_context(tc.tile_pool(name="v", bufs=1))
        qt_pool = ctx.enter_context(tc.tile_pool(name="qt", bufs=1))
        kt_pool = ctx.enter_context(tc.tile_pool(name="kt", bufs=1))

        # ---- constants
        ident_f = const_pool.tile([P, P], F32, name="ident_f")
        make_identity(nc, ident_f)
        ident_h = const_pool.tile([P, P], DT, name="ident_h")
        nc.vector.tensor_copy(ident_h[:], ident_f[:])
        # additive causal mask in matmul dtype: applied on the PE as an
        # extra accumulation (ident_h @ cmask_h) into the diagonal score
        # tile, so no other engine sits in the scores->exp chain.
        # -60000 is fp16-representable; after *scale and exp it's exactly 0.
        cmask_f = const_pool.tile([P, P], F32, name="cmask_f")
        make_causal_mask(nc, cmask_f, mask_val=-60000.0)
        # zero-padded wide mask: columns [NJ, NJ+P) hold the causal block;
        # slicing [NJ-dcol : NJ-dcol+n] puts the mask at the diagonal tile
        # and zeros everywhere else, covering the full psum chunk so the
        # accumulation stop flag finalizes the whole region.
        cmask_w = const_pool.tile([P, NJ + P], DT, name="cmask_w")
        nc.vector.memset(cmask_w[:], 0.0)
        nc.vector.tensor_copy(cmask_w[:, NJ:NJ + P], cmask_f[:])
        # bias tiles; DMAs are emitted later so they don't delay x/Qw
        qb_t = const_pool.tile([P, DC], F32, name="qb_t")
        kb_t = const_pool.tile([P, DC], F32, name="kb_t")
        vb_f = const_pool.tile([1, D], F32, name="vb_f")
        vb_bc = const_pool.tile([P, D], F32, name="vb_bc")

        # ---- resident tensors
        v_sb = [v_pool.tile([P, D], DT, name=f"v{i}") for i in range(RB)]
        qt_sb = [qt_pool.tile([P, C], DT, name=f"qt{d}") for d in range(DC)]
        kt_sb = [kt_pool.tile([P, C], DT, name=f"kt{d}") for d in range(DC)]

        def load_w(w_dram, pname, st_pool, w_pool, halves=False):
            """Stage a weight matrix into SBUF in DT.

            With halves=True the DMAs and casts are split into two D/2
            column groups, first group first, so projections over the first
            d-columns can start after only half the weight bytes land.
            Returns w_sb[half][e] tiles of [P, D/2] (or [P, D] if not).
            """
            if not halves:
                w_sb = []
                for e in range(EC):
                    wst = st_pool.tile([P, D], F32, tag="wst",
                                       name=f"{pname}st{e}")
                    nc.sync.dma_start(wst[:], w_dram[e * P:(e + 1) * P, :])
                    wt = w_pool.tile([P, D], DT, tag=f"w{e}",
                                     name=f"{pname}{e}")
                    nc.vector.tensor_copy(wt[:], wst[:])
                    w_sb.append(wt)
                return [w_sb]
            H = D // 2
            w_sb = [[], []]
            for h in range(2):
                for e in range(EC):
                    wst = st_pool.tile([P, H], F32, tag=f"wst{h}",
                                       name=f"{pname}st{h}_{e}")
                    nc.sync.dma_start(
                        wst[:], w_dram[e * P:(e + 1) * P, h * H:(h + 1) * H])
                    wt = w_pool.tile([P, H], DT, tag=f"w{h}_{e}",
                                     name=f"{pname}{h}_{e}")
                    nc.vector.tensor_copy(wt[:], wst[:])
                    w_sb[h].append(wt)
            return w_sb

        def proj_t_cc(w_sb, bias_t, dest, cc, ps_pool):
            """Projection into transposed layout for one c-chunk.

            w_sb is a list of column-group lists from load_w."""
            ngroups = len(w_sb)
            gw = D // ngroups // P          # d-chunks per group
            for dc in range(DC):
                g, dg = dc // gw, dc % gw
                ps = ps_pool.tile([P, CC], F32, tag="ps")
                for e in range(EC):
                    nc.tensor.matmul(
                        ps[:],
                        w_sb[g][e][:, dg * P:(dg + 1) * P],
                        xt[e][:, cc * CC:(cc + 1) * CC],
                        start=(e == 0), stop=(e == EC - 1))
                nc.scalar.activation(
                    dest[dc][:, cc * CC:(cc + 1) * CC], ps[:],
                    AF.Identity, bias=bias_t[:, dc:dc + 1])

        with tc.tile_pool(name="xt", bufs=1) as xt_pool:
            xt = [xt_pool.tile([P, C], DT, name=f"xt{e}") for e in range(EC)]

            # ---- P_X interleaved with P_Q (chunk-wise) so the PE has work
            # while Qw streams in. DMA order: x chunk 0 first, then Qw in
            # column halves, then biases, then the remaining x chunks.
            mark("px_pq")
            with tc.tile_pool(name="px_in", bufs=3) as xin_pool, \
                 tc.tile_pool(name="px_h", bufs=6) as xh_pool, \
                 tc.tile_pool(name="px_ps", bufs=4, space="PSUM") as pxps_pool, \
                 tc.tile_pool(name="pq_ps", bufs=4, space="PSUM") as pqps_pool, \
                 tc.tile_pool(name="pq_wst", bufs=3) as wstp, \
                 tc.tile_pool(name="pq_w", bufs=1) as wp:

                def px_cc(cc):
                    xhs = []
                    for cs in range(CC // P):
                        xrow = xin_pool.tile([P, E], F32, tag="xrow")
                        nc.sync.dma_start(
                            xrow[:],
                            x_d[cc * CC + cs * P: cc * CC + (cs + 1) * P, :])
                        xh = xh_pool.tile([P, E], DT, tag="xh")
                        nc.vector.tensor_copy(xh[:], xrow[:])
                        xhs.append(xh)
                    for e in range(EC):
                        pst = pxps_pool.tile([P, CC], DT, tag="pst")
                        for cs in range(CC // P):
                            nc.tensor.transpose(
                                pst[:, cs * P:(cs + 1) * P],
                                xhs[cs][:, e * P:(e + 1) * P], ident_h[:])
                        nc.scalar.copy(xt[e][:, cc * CC:(cc + 1) * CC], pst[:])

                px_cc(0)
                qw_sb = load_w(qw_d, "qw", wstp, wp, halves=True)
                nc.sync.dma_start(qb_t[:],
                                  qb_d.rearrange("(c p) -> p c", p=P))
                nc.sync.dma_start(kb_t[:],
                                  kb_d.rearrange("(c p) -> p c", p=P))
                for cc in range(NCC):
                    proj_t_cc(qw_sb, qb_t, qt_sb, cc, pqps_pool)
                    if cc + 1 < NCC:
                        px_cc(cc + 1)

            # ---- P_K
            mark("pk")
            with tc.tile_pool(name="pk_ps", bufs=4, space="PSUM") as ps_pool, \
                 tc.tile_pool(name="pk_wst", bufs=3) as wstp, \
                 tc.tile_pool(name="pk_w", bufs=1) as wp:
                kw_sb = load_w(kw_d, "kw", wstp, wp)
                for cc in range(NCC):
                    proj_t_cc(kw_sb, kb_t, kt_sb, cc, ps_pool)

            # ---- P_V (no bias; folded into epilogue)
            mark("pv")
            with tc.tile_pool(name="pv_ps", bufs=4, space="PSUM") as ps_pool, \
                 tc.tile_pool(name="pv_wst", bufs=3) as wstp, \
                 tc.tile_pool(name="pv_w", bufs=1) as wp:
                nc.sync.dma_start(vb_f[:], vb_d[None, :])
                nc.gpsimd.partition_broadcast(vb_bc[:], vb_f[0:1, :])
                vw_sb = load_w(vw_d, "vw", wstp, wp)[0]
                for ct in range(RB):
                    for dh in range(ND):
                        ps = ps_pool.tile([P, NJ], F32, tag="ps")
                        for e in range(EC):
                            nc.tensor.matmul(
                                ps[:], xt[e][:, ct * P:(ct + 1) * P],
                                vw_sb[e][:, dh * NJ:(dh + 1) * NJ],
                                start=(e == 0), stop=(e == EC - 1))
                        nc.vector.tensor_copy(
                            v_sb[ct][:, dh * NJ:(dh + 1) * NJ], ps[:])

        # ---- Phase A: causal attention, software-pipelined over row blocks
        mark("attn")
        knobs = knobs or {}
        with tc.tile_pool(name="e", bufs=knobs.get("e", 2)) as e_pool, \
             tc.tile_pool(name="et", bufs=knobs.get("et", 2)) as et_pool, \
             tc.tile_pool(name="r", bufs=knobs.get("r", 3)) as r_pool, \
             tc.tile_pool(name="os", bufs=knobs.get("os", 2)) as os_pool, \
             tc.tile_pool(name="a_s", bufs=knobs.get("s", 2),
                          space="PSUM") as s_pool, \
             tc.tile_pool(name="a_t", bufs=knobs.get("t", 2),
                          space="PSUM") as t_pool, \
             tc.tile_pool(name="a_o", bufs=knobs.get("o", 2),
                          space="PSUM") as o_pool:

            NS = knobs.get("NS", NJ)         # scores psum chunk width
            NSC = C // NS
            echunks = knobs.get("echunks", False)

            def emit_scores(i):
                """PE scores + ACT exp for row block i; returns state.

                The causal mask for the diagonal tile is an extra PE
                accumulation matmul (ident^T @ cmask == cmask), so the
                scores->exp chain involves no third engine.
                """
                ncols = (i + 1) * P
                njj = (ncols + NS - 1) // NS
                if echunks:
                    etile = [e_pool.tile([P, NS], DT, tag=f"E{jj}",
                                         name=f"E{jj}")
                             for jj in range(njj)]
                else:
                    etile = e_pool.tile([P, C], DT, tag="E")
                acc = r_pool.tile([P, NSC], F32, tag="acc")
                for jj in range(njj):
                    n = min(NS, ncols - jj * NS)
                    diag = jj == njj - 1
                    ps_s = s_pool.tile([P, NS], F32, tag="ps_s")
                    for d in range(DC):
                        nc.tensor.matmul(
                            ps_s[:, :n],
                            qt_sb[d][:, i * P:(i + 1) * P],
                            kt_sb[d][:, jj * NS:jj * NS + n],
                            start=(d == 0),
                            stop=(d == DC - 1 and not diag))
                    if diag:
                        dcol = i * P - jj * NS
                        s0 = NJ - dcol
                        nc.tensor.matmul(
                            ps_s[:, :n], ident_h[:], cmask_w[:, s0:s0 + n],
                            start=False, stop=True)
                    dst = (etile[jj][:, :n] if echunks
                           else etile[:, jj * NS:jj * NS + n])
                    nc.scalar.activation(
                        dst, ps_s[:, :n], AF.Exp,
                        scale=scale, accum_out=acc[:, jj:jj + 1])
                return etile, acc, njj

            def emit_denom(state):
                _, acc, njj = state
                rs = r_pool.tile([P, 1], F32, tag="rs")
                nc.vector.reduce_sum(rs[:], acc[:, :njj],
                                     axis=mybir.AxisListType.X)
                rinv = r_pool.tile([P, 1], F32, tag="rinv")
                nc.vector.reciprocal(rinv[:], rs[:])
                return rinv

            def emit_out(i, state, rinv):
                """PE transpose E + out matmuls + epilogue for row block i.

                Transposes of chunk jj+1 are interleaved between the out
                matmuls of chunk jj so the PE isn't waiting on the ACT
                psum->sbuf copy of the chunk it just transposed.
                """
                etile, _, _ = state
                ncols = (i + 1) * P
                njj = (ncols + NJ - 1) // NJ
                ettile = et_pool.tile([P, C], DT, tag="ET")

                def esrc(col, w):
                    if echunks:
                        return etile[col // NS][:, col % NS: col % NS + w]
                    return etile[:, col:col + w]

                def emit_t(jj):
                    n = min(NJ, ncols - jj * NJ)
                    ps_t = t_pool.tile([P, NJ], DT, tag="ps_t")
                    for j in range(n // P):
                        nc.tensor.transpose(
                            ps_t[:, j * P:(j + 1) * P],
                            esrc(jj * NJ + j * P, P), ident_h[:])
                    nc.vector.tensor_copy(ettile[:, jj * NJ:jj * NJ + n],
                                          ps_t[:, :n])

                emit_t(0)
                ps_o = [o_pool.tile([P, NJ], F32, tag=f"ps_o{dh}",
                                    name=f"ps_o{dh}")
                        for dh in range(ND)]
                for jj in range(njj):
                    if jj + 1 < njj:
                        emit_t(jj + 1)
                    j0 = jj * (NJ // P)
                    j1 = min((jj + 1) * (NJ // P), i + 1)
                    for dh in range(ND):
                        for j in range(j0, j1):
                            nc.tensor.matmul(
                                ps_o[dh][:],
                                ettile[:, j * P:(j + 1) * P],
                                v_sb[j][:, dh * NJ:(dh + 1) * NJ],
                                start=(j == 0), stop=(j == i))
                outst = os_pool.tile([P, D], F32, tag="outst")
                for dh in range(ND):
                    nc.vector.tensor_scalar_mul(
                        outst[:, dh * NJ:(dh + 1) * NJ], ps_o[dh][:], rinv[:])
                nc.vector.tensor_add(outst[:], outst[:], vb_bc[:])
                nc.sync.dma_start(out_d[i * P:(i + 1) * P, :], outst[:])

            prev = emit_scores(0)
            for i in range(1, RB):
                cur = emit_scores(i)
                rinv = emit_denom(prev)
                emit_out(i - 1, prev, rinv)
                prev = cur
            rinv = emit_denom(prev)
            emit_out(RB - 1, prev, rinv)
        mark("end")

    nc.compile()
    return nc


_CACHE = {}


def _built(C=2048, E=1024, D=1024, n_cores=8, loop=1, dt="fp16"):
    key = (C, E, D, n_cores, loop, dt)
    if key not in _CACHE:
        _CACHE[key] = build(C, E, D, n_cores, loop, dt)
    return _CACHE[key]


def _executable(C=2048, E=1024, D=1024, n_cores=8, loop=1, dt="fp16"):
    """Cached jitted SPMD executable for the built Bass module.

    Replicates concourse.bass2jax.run_bass_via_pjrt's multi-core path but
    caches the jit so repeat calls don't retrace, and exposes the pieces
    needed for device-resident benchmarking.
    """
    key = ("exec", C, E, D, n_cores, loop, dt)
    if key in _CACHE:
        return _CACHE[key]
    import jax
    from jax.sharding import Mesh, PartitionSpec
    from jax.experimental.shard_map import shard_map
    from concourse import bass2jax, mybir as _mybir

    nc = _built(C, E, D, n_cores, loop, dt)
    bass2jax.install_neuronx_cc_hook()

    partition_name = (nc.partition_id_tensor.name
                      if nc.partition_id_tensor else None)
    in_names, out_names, out_avals, zero_outs = [], [], [], []
    for alloc in nc.m.functions[0].allocations:
        if not isinstance(alloc, _mybir.MemoryLocationSet):
            continue
        name = alloc.memorylocations[0].name
        if alloc.kind == "ExternalInput":
            if name != partition_name:
                in_names.append(name)
        elif alloc.kind == "ExternalOutput":
            out_names.append(name)
            shape = tuple(alloc.tensor_shape)
            dtype = _mybir.dt.np(alloc.dtype)
            out_avals.append(jax.core.ShapedArray(shape, dtype))
            zero_outs.append(np.zeros(shape, dtype))
    n_params = len(in_names)
    all_names = in_names + out_names
    if partition_name is not None:
        all_names = all_names + [partition_name]

    def _body(*args):
        operands = list(args)
        if partition_name is not None:
            operands.append(bass2jax.partition_id_tensor())
        outs = bass2jax._bass_exec_p.bind(
            *operands,
            out_avals=tuple(out_avals),
            in_names=tuple(all_names),
            out_names=tuple(out_names),
            lowering_input_output_aliases=(),
            sim_require_finite=True,
            sim_require_nnan=True,
            nc=nc,
        )
        return tuple(outs)

    devices = jax.devices()[:n_cores]
    mesh = Mesh(np.asarray(devices), ("core",))
    n_outs = len(out_names)
    sharded = jax.jit(
        shard_map(_body, mesh=mesh,
                  in_specs=(PartitionSpec("core"),) * (n_params + n_outs),
                  out_specs=(PartitionSpec("core"),) * n_outs,
                  check_rep=False),
        donate_argnums=tuple(range(n_params, n_params + n_outs)),
        keep_unused=True,
    )
    res = dict(fn=sharded, in_names=in_names, out_names=out_names,
               out_avals=out_avals, zero_outs=zero_outs, mesh=mesh,
               n_cores=n_cores)
    _CACHE[key] = res
    return res


def run(inputs, C=2048, E=1024, D=1024, n_cores=8, dt="fp16"):
    ex = _executable(C, E, D, n_cores, 1, dt)
    B = inputs["x"].shape[0]
    assert B == n_cores
    f = lambda a: np.ascontiguousarray(np.asarray(a, dtype=np.float32))
    shared = {k: f(inputs[k]) for k in ("Qw", "Qb", "Kw", "Kb", "Vw", "Vb")}
    x = f(inputs["x"])
    per_core = [dict(x=x[b], **shared) for b in range(B)]
    concat_in = [
        np.concatenate([per_core[c][n] for c in range(n_cores)], axis=0)
        for n in ex["in_names"]
    ]
    concat_zeros = [
        np.zeros((n_cores * z.shape[0], *z.shape[1:]), z.dtype)
        for z in ex["zero_outs"]
    ]
    out_arrs = ex["fn"](*concat_in, *concat_zeros)
    i = ex["out_names"].index("out")
    out = np.asarray(out_arrs[i]).reshape(n_cores, *ex["out_avals"][i].shape)
    return out


def kernel(**inputs) -> np.ndarray:
    return run(inputs)


# revision 36
# speedup vs baseline: 1.0610x; 1.0610x over previous
"""Causal single-head attention on 8 trn2 NeuronCores, data-parallel over batch.

Per core (one batch element, C=2048 ctx, E=1024 emb, D=1024 query_dim):
  P_X: cast x to fp16 (DVE), PE-transpose fp16 -> xt resident SBUF,
       interleaved chunk-wise with P_Q so the PE never idles at startup.
  P_Q: qT = (Qw^T @ x^T) + Qb   -> resident SBUF (DT), [d-chunk][128, C].
  P_K: kT = (Kw^T @ x^T) + Kb   -> resident SBUF (DT), same layout.
  P_V: v  = (x @ Vw)            -> resident SBUF (DT), natural layout;
       Vb is folded into the epilogue: out = (E^T V)/rowsum + Vb.
  A:   software-pipelined over 128-row query blocks: scores_{i+1} is issued
       before transpose_i/out_i so the PE never waits on the scalar-engine
       exp. Per block i: scores = qT_i^T @ kT (causal range only), additive
       -1e9 mask on the diagonal tile, E = exp(scale*scores) with fused
       row-sum (ACT), PE-transpose E, psum->sbuf E^T copies on ACT,
       out = sum_j E^T_j @ v_j in PSUM, epilogue: DVE scale by 1/rowsum,
       GPSIMD adds broadcast Vb, DMA out.
"""

import os
import sys

for _p in ("/opt/trn_rl_repo", "/root/.axon_site/_ro/trn_rl_repo"):
    if os.path.isdir(_p) and _p not in sys.path:
        sys.path.insert(0, _p)

from contextlib import ExitStack

import numpy as np

import concourse.bass as bass
import concourse.tile as tile
from concourse import bacc, mybir
from concourse.masks import make_causal_mask, make_identity

F32 = mybir.dt.float32
AF = mybir.ActivationFunctionType
DTYPES = {"fp16": mybir.dt.float16, "bf16": mybir.dt.bfloat16}

P = 128


def build(C=2048, E=1024, D=1024, n_cores=8, loop=1, dt="fp16", marks=None,
          knobs=None):
    DT = DTYPES[dt]
    CC = 512            # c-chunk width for projection passes
    NJ = 512            # matmul moving-dim width
    NCC = C // CC
    EC = E // P         # contraction chunks for projections
    DC = D // P
    RB = C // P         # number of 128-row blocks
    ND = D // NJ
    NKC = C // NJ
    scale = float(D) ** -0.5

    knobs = knobs or {}
    phases = knobs.get("phases", "xqkva")
    nc = bacc.Bacc("TRN2", target_bir_lowering=False, debug=False,
                   num_devices=n_cores)
    x_d = nc.dram_tensor("x", [C, E], F32, kind="ExternalInput").ap()
    qw_d = nc.dram_tensor("Qw", [E, D], F32, kind="ExternalInput").ap()
    qb_d = nc.dram_tensor("Qb", [D], F32, kind="ExternalInput").ap()
    kw_d = nc.dram_tensor("Kw", [E, D], F32, kind="ExternalInput").ap()
    kb_d = nc.dram_tensor("Kb", [D], F32, kind="ExternalInput").ap()
    vw_d = nc.dram_tensor("Vw", [E, D], F32, kind="ExternalInput").ap()
    vb_d = nc.dram_tensor("Vb", [D], F32, kind="ExternalInput").ap()
    out_d = nc.dram_tensor("out", [C, D], F32, kind="ExternalOutput").ap()

    def mark(label):
        if marks is not None:
            marks.append((label, nc.next_id()))

    with tile.TileContext(nc) as tc, ExitStack() as ctx:
        if loop > 1:
            ctx.enter_context(tc.For_i(0, loop, 1))
        const_pool = ctx.enter_context(tc.tile_pool(name="const", bufs=1))
        v_pool = ctx.enter_context(tc.tile_pool(name="v", bufs=1))
        qt_pool = ctx.enter_context(tc.tile_pool(name="qt", bufs=1))
        kt_pool = ctx.enter_context(tc.tile_pool(name="kt", bufs=1))

        # ---- constants
        ident_f = const_pool.tile([P, P], F32, name="ident_f")
        make_identity(nc, ident_f)
        ident_h = const_pool.tile([P, P], DT, name="ident_h")
        nc.vector.tensor_copy(ident_h[:], ident_f[:])
        # additive causal mask in matmul dtype: applied on the PE as an
        # extra accumulation (ident_h @ cmask_h) into the diagonal score
        # tile, so no other engine sits in the scores->exp chain.
        # -60000 is fp16-representable; after *scale and exp it's exactly 0.
        cmask_f = const_pool.tile([P, P], F32, name="cmask_f")
        make_causal_mask(nc, cmask_f, mask_val=-60000.0)
        # zero-padded wide mask: columns [NJ, NJ+P) hold the causal block;
        # slicing [NJ-dcol : NJ-dcol+n] puts the mask at the diagonal tile
        # and zeros everywhere else, covering the full psum chunk so the
        # accumulation stop flag finalizes the whole region.
        cmask_w = const_pool.tile([P, NJ + P], DT, name="cmask_w")
        nc.vector.memset(cmask_w[:], 0.0)
        nc.vector.tensor_copy(cmask_w[:, NJ:NJ + P], cmask_f[:])
        # bias tiles; DMAs are emitted later so they don't delay x/Qw
        qb_t = const_pool.tile([P, DC], F32, name="qb_t")
        kb_t = const_pool.tile([P, DC], F32, name="kb_t")
        vb_f = const_pool.tile([1, D], F32, name="vb_f")
        vb_bc = const_pool.tile([P, D], F32, name="vb_bc")

        # ---- resident tensors
        v_sb = [v_pool.tile([P, D], DT, name=f"v{i}") for i in range(RB)]
        qt_sb = [qt_pool.tile([P, C], DT, name=f"qt{d}") for d in range(DC)]
        kt_sb = [kt_pool.tile([P, C], DT, name=f"kt{d}") for d in range(DC)]

        def load_w(w_dram, pname, st_pool, w_pool, halves=False):
            """Stage a weight matrix into SBUF in DT.

            With halves=True the DMAs and casts are split into two D/2
            column groups, first group first, so projections over the first
            d-columns can start after only half the weight bytes land.
            Returns w_sb[half][e] tiles of [P, D/2] (or [P, D] if not).
            """
            if not halves:
                w_sb = []
                for e in range(EC):
                    wst = st_pool.tile([P, D], F32, tag="wst",
                                       name=f"{pname}st{e}")
                    nc.sync.dma_start(wst[:], w_dram[e * P:(e + 1) * P, :])
                    wt = w_pool.tile([P, D], DT, tag=f"w{e}",
                                     name=f"{pname}{e}")
                    nc.vector.tensor_copy(wt[:], wst[:])
                    w_sb.append(wt)
                return [w_sb]
            H = D // 2
            w_sb = [[], []]
            for h in range(2):
                for e in range(EC):
                    wst = st_pool.tile([P, H], F32, tag=f"wst{h}",
                                       name=f"{pname}st{h}_{e}")
                    nc.sync.dma_start(
                        wst[:], w_dram[e * P:(e + 1) * P, h * H:(h + 1) * H])
                    wt = w_pool.tile([P, H], DT, tag=f"w{h}_{e}",
                                     name=f"{pname}{h}_{e}")
                    nc.vector.tensor_copy(wt[:], wst[:])
                    w_sb[h].append(wt)
            return w_sb

        def proj_t_cc(w_sb, bias_t, dest, cc, ps_pool):
            """Projection into transposed layout for one c-chunk.

            w_sb is a list of column-group lists from load_w."""
            ngroups = len(w_sb)
            gw = D // ngroups // P          # d-chunks per group
            for dc in range(DC):
                g, dg = dc // gw, dc % gw
                ps = ps_pool.tile([P, CC], F32, tag="ps")
                for e in range(EC):
                    nc.tensor.matmul(
                        ps[:],
                        w_sb[g][e][:, dg * P:(dg + 1) * P],
                        xt[e][:, cc * CC:(cc + 1) * CC],
                        start=(e == 0), stop=(e == EC - 1))
                nc.scalar.activation(
                    dest[dc][:, cc * CC:(cc + 1) * CC], ps[:],
                    AF.Identity, bias=bias_t[:, dc:dc + 1])

        if "x" not in phases:
            # attention-only probe: fill residents with a constant
            for tl in qt_sb + kt_sb + v_sb:
                nc.gpsimd.memset(tl[:], 0.01)
            nc.gpsimd.memset(vb_bc[:], 0.0)

        if "x" in phases:
         with tc.tile_pool(name="xt", bufs=1) as xt_pool:
            xt = [xt_pool.tile([P, C], DT, name=f"xt{e}") for e in range(EC)]

            # ---- P_X interleaved with P_Q (chunk-wise) so the PE has work
            # while Qw streams in. DMA order: x chunk 0 first, then Qw in
            # column halves, then biases, then the remaining x chunks.
            mark("px_pq")
            with tc.tile_pool(name="px_in", bufs=3) as xin_pool, \
                 tc.tile_pool(name="px_h", bufs=6) as xh_pool, \
                 tc.tile_pool(name="px_ps", bufs=4, space="PSUM") as pxps_pool, \
                 tc.tile_pool(name="pq_ps", bufs=4, space="PSUM") as pqps_pool, \
                 tc.tile_pool(name="pq_wst", bufs=3) as wstp, \
                 tc.tile_pool(name="pq_w", bufs=1) as wp:

                def px_cc(cc):
                    xhs = []
                    for cs in range(CC // P):
                        xrow = xin_pool.tile([P, E], F32, tag="xrow")
                        nc.sync.dma_start(
                            xrow[:],
                            x_d[cc * CC + cs * P: cc * CC + (cs + 1) * P, :])
                        xh = xh_pool.tile([P, E], DT, tag="xh")
                        nc.vector.tensor_copy(xh[:], xrow[:])
                        xhs.append(xh)
                    for e in range(EC):
                        pst = pxps_pool.tile([P, CC], DT, tag="pst")
                        for cs in range(CC // P):
                            nc.tensor.transpose(
                                pst[:, cs * P:(cs + 1) * P],
                                xhs[cs][:, e * P:(e + 1) * P], ident_h[:])
                        nc.scalar.copy(xt[e][:, cc * CC:(cc + 1) * CC], pst[:])

                def load_bias_t(b_dram, dest, brow):
                    """[D] dram -> [P, DC] sbuf via one contiguous DMA of
                    [DC, P] (DC descriptors) and a tiny PE transpose, instead
                    of a 1024-descriptor strided DMA (which measures ~1ms of
                    DMA-queue time on hw)."""
                    nc.sync.dma_start(
                        brow[:], b_dram.rearrange("(c p) -> c p", p=P))
                    ps = pqps_pool.tile([P, DC], F32, tag="ps", name="ps_b")
                    nc.tensor.transpose(ps[:], brow[:], ident_f[:DC, :DC])
                    nc.scalar.copy(dest[:], ps[:])

                px_cc(0)
                if "q" in phases:
                    qw_sb = load_w(qw_d, "qw", wstp, wp, halves=True)
                    if knobs.get("nobias"):
                        nc.vector.memset(qb_t[:], 0.0)
                        nc.vector.memset(kb_t[:], 0.0)
                    else:
                        qb_row = const_pool.tile([DC, P], F32, name="qb_row")
                        kb_row = const_pool.tile([DC, P], F32, name="kb_row")
                        load_bias_t(qb_d, qb_t, qb_row)
                        load_bias_t(kb_d, kb_t, kb_row)
                for cc in range(NCC):
                    if "q" in phases:
                        proj_t_cc(qw_sb, qb_t, qt_sb, cc, pqps_pool)
                    if cc + 1 < NCC:
                        px_cc(cc + 1)

            # ---- P_K
            mark("pk")
            if "k" in phases:
                with tc.tile_pool(name="pk_ps", bufs=4,
                                  space="PSUM") as ps_pool, \
                     tc.tile_pool(name="pk_wst", bufs=3) as wstp, \
                     tc.tile_pool(name="pk_w", bufs=1) as wp:
                    kw_sb = load_w(kw_d, "kw", wstp, wp)
                    for cc in range(NCC):
                        proj_t_cc(kw_sb, kb_t, kt_sb, cc, ps_pool)

            # ---- P_V (no bias; folded into epilogue)
            mark("pv")
            if "v" in phases:
                with tc.tile_pool(name="pv_ps", bufs=4,
                                  space="PSUM") as ps_pool, \
                     tc.tile_pool(name="pv_wst", bufs=3) as wstp, \
                     tc.tile_pool(name="pv_w", bufs=1) as wp:
                    nc.sync.dma_start(vb_f[:], vb_d[None, :])
                    nc.gpsimd.partition_broadcast(vb_bc[:], vb_f[0:1, :])
                    vw_sb = load_w(vw_d, "vw", wstp, wp)[0]
                    for ct in range(RB):
                        for dh in range(ND):
                            ps = ps_pool.tile([P, NJ], F32, tag="ps")
                            for e in range(EC):
                                nc.tensor.matmul(
                                    ps[:], xt[e][:, ct * P:(ct + 1) * P],
                                    vw_sb[e][:, dh * NJ:(dh + 1) * NJ],
                                    start=(e == 0), stop=(e == EC - 1))
                            nc.vector.tensor_copy(
                                v_sb[ct][:, dh * NJ:(dh + 1) * NJ], ps[:])

        # ---- Phase A: causal attention, software-pipelined over row blocks
        mark("attn")
        with tc.tile_pool(name="e", bufs=knobs.get("e", 2)) as e_pool, \
             tc.tile_pool(name="et", bufs=knobs.get("et", 2)) as et_pool, \
             tc.tile_pool(name="r", bufs=knobs.get("r", 3)) as r_pool, \
             tc.tile_pool(name="os", bufs=knobs.get("os", 2)) as os_pool, \
             tc.tile_pool(name="a_s", bufs=knobs.get("s", 2),
                          space="PSUM") as s_pool, \
             tc.tile_pool(name="a_t", bufs=knobs.get("t", 2),
                          space="PSUM") as t_pool, \
             tc.tile_pool(name="a_o", bufs=knobs.get("o", 2),
                          space="PSUM") as o_pool:

            NS = knobs.get("NS", NJ)         # scores psum chunk width
            NSC = C // NS
            echunks = knobs.get("echunks", False)

            def emit_scores(i):
                """PE scores + ACT exp for row block i; returns state.

                The causal mask for the diagonal tile is an extra PE
                accumulation matmul (ident^T @ cmask == cmask), so the
                scores->exp chain involves no third engine.
                """
                ncols = (i + 1) * P
                njj = (ncols + NS - 1) // NS
                if echunks:
                    etile = [e_pool.tile([P, NS], DT, tag=f"E{jj}",
                                         name=f"E{jj}")
                             for jj in range(njj)]
                else:
                    etile = e_pool.tile([P, C], DT, tag="E")
                acc = r_pool.tile([P, NSC], F32, tag="acc")
                for jj in range(njj):
                    n = min(NS, ncols - jj * NS)
                    diag = jj == njj - 1
                    ps_s = s_pool.tile([P, NS], F32, tag="ps_s")
                    for d in range(DC):
                        nc.tensor.matmul(
                            ps_s[:, :n],
                            qt_sb[d][:, i * P:(i + 1) * P],
                            kt_sb[d][:, jj * NS:jj * NS + n],
                            start=(d == 0),
                            stop=(d == DC - 1 and not diag))
                    if diag:
                        dcol = i * P - jj * NS
                        s0 = NJ - dcol
                        nc.tensor.matmul(
                            ps_s[:, :n], ident_h[:], cmask_w[:, s0:s0 + n],
                            start=False, stop=True)
                    dst = (etile[jj][:, :n] if echunks
                           else etile[:, jj * NS:jj * NS + n])
                    if knobs.get("noacc"):
                        nc.scalar.activation(dst, ps_s[:, :n], AF.Exp,
                                             scale=scale)
                    else:
                        nc.scalar.activation(
                            dst, ps_s[:, :n], AF.Exp,
                            scale=scale, accum_out=acc[:, jj:jj + 1])
                return etile, acc, njj

            def emit_denom(state):
                """Row-sum of the per-chunk accumulators.

                The reduce runs on ACT (same engine as the exps that write
                acc) so acc never has a cross-engine WAR — otherwise Tile
                batches the exps' waits into a per-block EventSemaphore that
                serializes the whole block's exps behind its last scores
                chunk (~2.5us/block PE stall).
                """
                _, acc, njj = state
                rinv = r_pool.tile([P, 1], F32, tag="rinv")
                if knobs.get("noacc"):
                    nc.vector.memset(rinv[:], 1.0)
                    return rinv
                rs = r_pool.tile([P, 1], F32, tag="rs")
                scr = r_pool.tile([P, NSC], F32, tag="scr")
                nc.scalar.activation(scr[:, :njj], acc[:, :njj],
                                     AF.Identity, accum_out=rs[:])
                nc.vector.reciprocal(rinv[:], rs[:])
                return rinv

            def emit_out(i, state, rinv):
                """PE transpose E + out matmuls + epilogue for row block i.

                Transposes of chunk jj+1 are interleaved between the out
                matmuls of chunk jj so the PE isn't waiting on the ACT
                psum->sbuf copy of the chunk it just transposed.
                """
                etile, _, _ = state
                ncols = (i + 1) * P
                njj = (ncols + NJ - 1) // NJ
                ettile = et_pool.tile([P, C], DT, tag="ET")

                def esrc(col, w):
                    if echunks:
                        return etile[col // NS][:, col % NS: col % NS + w]
                    return etile[:, col:col + w]

                def emit_t(jj):
                    n = min(NJ, ncols - jj * NJ)
                    ps_t = t_pool.tile([P, NJ], DT, tag="ps_t")
                    for j in range(n // P):
                        nc.tensor.transpose(
                            ps_t[:, j * P:(j + 1) * P],
                            esrc(jj * NJ + j * P, P), ident_h[:])
                    nc.vector.tensor_copy(ettile[:, jj * NJ:jj * NJ + n],
                                          ps_t[:, :n])

                emit_t(0)
                ps_o = [o_pool.tile([P, NJ], F32, tag=f"ps_o{dh}",
                                    name=f"ps_o{dh}")
                        for dh in range(ND)]
                for jj in range(njj):
                    if jj + 1 < njj:
                        emit_t(jj + 1)
                    j0 = jj * (NJ // P)
                    j1 = min((jj + 1) * (NJ // P), i + 1)
                    for dh in range(ND):
                        for j in range(j0, j1):
                            nc.tensor.matmul(
                                ps_o[dh][:],
                                ettile[:, j * P:(j + 1) * P],
                                v_sb[j][:, dh * NJ:(dh + 1) * NJ],
                                start=(j == 0), stop=(j == i))
                outst = os_pool.tile([P, D], F32, tag="outst")
                for dh in range(ND):
                    nc.vector.tensor_scalar_mul(
                        outst[:, dh * NJ:(dh + 1) * NJ], ps_o[dh][:], rinv[:])
                nc.vector.tensor_add(outst[:], outst[:], vb_bc[:])
                nc.sync.dma_start(out_d[i * P:(i + 1) * P, :], outst[:])

            if "a" in phases:
                prev = emit_scores(0)
                for i in range(1, RB):
                    cur = emit_scores(i)
                    rinv = emit_denom(prev)
                    emit_out(i - 1, prev, rinv)
                    prev = cur
                rinv = emit_denom(prev)
                emit_out(RB - 1, prev, rinv)
            else:
                outst = os_pool.tile([P, D], F32, tag="outst")
                nc.vector.memset(outst[:], 0.0)
                nc.sync.dma_start(out_d[0:P, :], outst[:])
        mark("end")

    nc.compile()
    return nc


_CACHE = {}


def _built(C=2048, E=1024, D=1024, n_cores=8, loop=1, dt="fp16", knobs=None):
    key = (C, E, D, n_cores, loop, dt,
           tuple(sorted((knobs or {}).items())))
    if key not in _CACHE:
        _CACHE[key] = build(C, E, D, n_cores, loop, dt, knobs=knobs)
    return _CACHE[key]


def _executable(C=2048, E=1024, D=1024, n_cores=8, loop=1, dt="fp16",
                knobs=None):
    """Cached jitted SPMD executable for the built Bass module.

    Replicates concourse.bass2jax.run_bass_via_pjrt's multi-core path but
    caches the jit so repeat calls don't retrace, and exposes the pieces
    needed for device-resident benchmarking.
    """
    key = ("exec", C, E, D, n_cores, loop, dt,
           tuple(sorted((knobs or {}).items())))
    if key in _CACHE:
        return _CACHE[key]
    import jax
    from jax.sharding import Mesh, PartitionSpec
    from jax.experimental.shard_map import shard_map
    from concourse import bass2jax, mybir as _mybir

    nc = _built(C, E, D, n_cores, loop, dt, knobs=knobs)
    bass2jax.install_neuronx_cc_hook()

    partition_name = (nc.partition_id_tensor.name
                      if nc.partition_id_tensor else None)
    in_names, out_names, out_avals, zero_outs = [], [], [], []
    for alloc in nc.m.functions[0].allocations:
        if not isinstance(alloc, _mybir.MemoryLocationSet):
            continue
        name = alloc.memorylocations[0].name
        if alloc.kind == "ExternalInput":
            if name != partition_name:
                in_names.append(name)
        elif alloc.kind == "ExternalOutput":
            out_names.append(name)
            shape = tuple(alloc.tensor_shape)
            dtype = _mybir.dt.np(alloc.dtype)
            out_avals.append(jax.core.ShapedArray(shape, dtype))
            zero_outs.append(np.zeros(shape, dtype))
    n_params = len(in_names)
    all_names = in_names + out_names
    if partition_name is not None:
        all_names = all_names + [partition_name]

    def _body(*args):
        operands = list(args)
        if partition_name is not None:
            operands.append(bass2jax.partition_id_tensor())
        outs = bass2jax._bass_exec_p.bind(
            *operands,
            out_avals=tuple(out_avals),
            in_names=tuple(all_names),
            out_names=tuple(out_names),
            lowering_input_output_aliases=(),
            sim_require_finite=True,
            sim_require_nnan=True,
            nc=nc,
        )
        return tuple(outs)

    devices = jax.devices()[:n_cores]
    mesh = Mesh(np.asarray(devices), ("core",))
    n_outs = len(out_names)
    sharded = jax.jit(
        shard_map(_body, mesh=mesh,
                  in_specs=(PartitionSpec("core"),) * (n_params + n_outs),
                  out_specs=(PartitionSpec("core"),) * n_outs,
                  check_rep=False),
        donate_argnums=tuple(range(n_params, n_params + n_outs)),
        keep_unused=True,
    )
    res = dict(fn=sharded, in_names=in_names, out_names=out_names,
               out_avals=out_avals, zero_outs=zero_outs, mesh=mesh,
               n_cores=n_cores)
    _CACHE[key] = res
    return res


def run(inputs, C=2048, E=1024, D=1024, n_cores=8, dt="fp16"):
    ex = _executable(C, E, D, n_cores, 1, dt)
    B = inputs["x"].shape[0]
    assert B == n_cores
    f = lambda a: np.ascontiguousarray(np.asarray(a, dtype=np.float32))
    shared = {k: f(inputs[k]) for k in ("Qw", "Qb", "Kw", "Kb", "Vw", "Vb")}
    x = f(inputs["x"])
    per_core = [dict(x=x[b], **shared) for b in range(B)]
    concat_in = [
        np.concatenate([per_core[c][n] for c in range(n_cores)], axis=0)
        for n in ex["in_names"]
    ]
    concat_zeros = [
        np.zeros((n_cores * z.shape[0], *z.shape[1:]), z.dtype)
        for z in ex["zero_outs"]
    ]
    out_arrs = ex["fn"](*concat_in, *concat_zeros)
    i = ex["out_names"].index("out")
    out = np.asarray(out_arrs[i]).reshape(n_cores, *ex["out_avals"][i].shape)
    return out


def kernel(**inputs) -> np.ndarray:
    return run(inputs)
